# revision 3
# baseline (speedup 1.0000x reference)
"""DualAttention Trainium2 kernel.

Problem: x:[2,64,20,20,20]; three separable 1-D convs produce q0 (H-axis),
k0 (D-axis), v (W-axis), each [B,C,N] with N=8000; scores = k0^T q0 [B,N,N];
softmax over the key axis i (axis 1); out = v @ attn, reshaped back.

Sharding: 8 cores = 2 batches x 4 query-column slices of 2000. Each core
computes full k0/v (cheap convs) and its q0 slice, then a flash-style
scores->exp->accumulate loop. No collectives.

Per-core device algorithm (Tile framework), v2 (trace-driven rewrite):
  - all conv inputs/weights bf16 (full-rate PE, half the DMA bytes).
  - D-axis taps of the k conv are +/-400-column OFFSET slices of xin
    (valid region contiguous in flattened N) -> no host-stacked xD tensor.
  - biases folded into the conv matmuls via a ones row (K=65) in
    xin/xq; the k bias is dropped entirely (adds a per-query-column
    constant to scores -> cancels in softmax over the key axis).
  - conv PSUM->SBUF casts alternate ACT/DVE.
  - ~28 dummy warmup matmuls issued first each iteration keep the PE HAM
    un-throttled through the input-DMA window (conv MMs otherwise run at
    the cold 1.2 GHz isolated-MM latency).
  - scores: bf16 k0/q0 (fp32 PSUM); i-tile pairs alternate PE row groups
    (k0/q0 duplicated across partition halves) -> adjacent tiles stream
    concurrently (verified in trace: paired slices sum to one chunk time).
  - exp with bias -8 (uniform shift cancels in softmax), bf16 out; half
    of i-tiles on ScalarE (exact), half on VectorE via a Schraudolph-style
    int16 tensor_scalar whose bits are the bf16 exp approximation.
  - vT built with identity matmuls, 8 transposes packed per PSUM bank,
    one DVE cast per group.
  - second matmul: single K=128 MM per i-tile (lhsT = [v^T | 1] bf16)
    accumulating numerator+denominator into ONE psum bank (same PE
    cycles as split halves, frees 2 banks -> 3 score buffers).
  - normalize: reciprocal_approx_fast on the den row (~5x faster than
    reciprocal), gpsimd partition-broadcast, fused multiply, per-pass
    output DMA.
"""

import math
import numpy as np
import ml_dtypes

B, C, D, H, W = 2, 64, 20, 20, 20
N = D * H * W            # 8000
NCORES = 8
CPB = 4                  # cores per batch
MS = N // CPB            # 2000 query columns per core
ITILE = 128
NFULL = N // ITILE       # 62 full i-tiles
TAILW = N - NFULL * ITILE  # 64
NT = NFULL + 1           # 63
M_PASSES = [(0, 1024), (1024, 976)]
EXP_BIAS = -8.0
DSH = H * W              # 400: one D step in flattened N
NPB = np.dtype(ml_dtypes.bfloat16)

# Schraudolph-style exp on DVE: int16 bits = SCH_A*s + SCH_B viewed as bf16.
# bf16 bits = 128*log2(v) + 16256; log2(e^(s+EXP_BIAS)) = (s+EXP_BIAS)/ln2.
SCH_A = 128.0 / math.log(2.0)
SCH_C = 7.4
SCH_B = 16256.0 - SCH_C + SCH_A * EXP_BIAS

# build-time config knobs (A/B testing)
CONFIG = {
    "dve_res": (1, 3, 5, 7, 9),  # t%10 in set -> exp on DVE (50%)
    "warm_mms": 28,
    "hoist": 6,
    "mp_bufs": 3,
    "ep_bufs": 12,
    "hint": True,
}

_CACHE = {}

# weight-pack column offsets (all blocks 64 wide)
WKC, WKP, WKM, WQC, WQH, WVC, WVW = 0, 64, 128, 192, 256, 320, 384
WCOLS = 448


def _build_bass(reps=1, bench_loop=0, passes=None):
    if passes is None:
        passes = M_PASSES
    import contextlib
    import concourse.tile as tile
    import concourse.mybir as mybir
    from concourse import bacc
    from concourse.masks import make_identity

    F32 = mybir.dt.float32
    I16 = mybir.dt.int16
    BF16 = mybir.dt.bfloat16
    EXP = mybir.ActivationFunctionType.Exp
    IDENT = mybir.ActivationFunctionType.Identity
    MULT = mybir.AluOpType.mult
    ADD = mybir.AluOpType.add
    ET = mybir.EngineType

    nc = bacc.Bacc("TRN2", target_bir_lowering=False, debug=False,
                   num_devices=NCORES)

    xin_d = nc.dram_tensor("xin", [65, N], BF16, kind="ExternalInput")
    xW_d = nc.dram_tensor("xW", [128, N], BF16, kind="ExternalInput")
    xH_d = nc.dram_tensor("xH", [128, MS], BF16, kind="ExternalInput")
    xq_d = nc.dram_tensor("xq", [65, MS], BF16, kind="ExternalInput")
    wp_d = nc.dram_tensor("wp", [128, WCOLS], BF16, kind="ExternalInput")
    out_d = nc.dram_tensor("out", [64, MS], F32, kind="ExternalOutput")

    PIECE = 2048  # input DMA piece size for pipelining

    with tile.TileContext(nc) as tc:
        hint = tuple()
        if bench_loop and CONFIG["hint"]:
            hint = (ET.PE, ET.Activation, ET.DVE)
        loop_cm = (tc.For_i(0, bench_loop, 1, hint_engines=hint)
                   if bench_loop else contextlib.nullcontext())
        with loop_cm:
         for _rep in range(reps):
            with tc.tile_pool(name="persist", bufs=1) as per, \
                 tc.tile_pool(name="mps", bufs=CONFIG["mp_bufs"],
                              space="PSUM") as mp, \
                 tc.tile_pool(name="ops", bufs=1, space="PSUM") as opp, \
                 tc.tile_pool(name="eps", bufs=CONFIG["ep_bufs"]) as ep, \
                 tc.tile_pool(name="fin", bufs=1) as fin, \
                 tc.tile_pool(name="prep", bufs=1) as pr:

                w_sb = per.tile([128, WCOLS], BF16, name="w_sb")
                nc.gpsimd.dma_start(w_sb[:, :], wp_d.ap())
                k0_sb = per.tile([128, N], BF16, name="k0_sb")
                q0_sb = per.tile([128, MS], BF16, name="q0_sb")
                v_sb = per.tile([64, N], BF16, name="v_sb")
                vT_sb = per.tile([128, NT * 65], BF16, name="vT_sb")
                out_sb = per.tile([64, MS], F32, name="out_sb")
                identb = per.tile([64, 64], BF16, name="identb")
                expb = per.tile([128, 1], F32, name="expb")
                wsrc = per.tile([64, 512], BF16, name="wsrc")

                make_identity(nc, identb[:, :])
                nc.vector.memset(expb[:, :], EXP_BIAS)
                nc.vector.memset(vT_sb[:, :], 1.0)
                nc.gpsimd.memset(wsrc[:, :], 0.0)
                vT_view = vT_sb[:, :].rearrange("p (t c) -> p t c", c=65)

                # --- PE warmup: keep HAM un-throttled during input DMA ---
                warm = mp.tile([64, 512], F32, tag="s", name="warm")
                for _ in range(CONFIG["warm_mms"]):
                    nc.tensor.matmul(warm[0:64, :], lhsT=wsrc[0:64, 0:64],
                                     rhs=wsrc[0:64, :],
                                     start=True, stop=True)

                copy_tick = [0]

                def psum_to_sbuf(dst, src):
                    # alternate ACT/DVE for PSUM->SBUF conv casts
                    if copy_tick[0] % 2 == 0:
                        nc.scalar.activation(dst, src, IDENT)
                    else:
                        nc.vector.tensor_copy(out=dst, in_=src)
                    copy_tick[0] += 1

                # --- q conv first (small inputs, unblocks scores) ---
                xH_sb = pr.tile([128, MS], BF16, tag="xh", name="xH_sb")
                nc.sync.dma_start(xH_sb[:, :], xH_d.ap())
                xq_sb = pr.tile([65, MS], BF16, tag="xq", name="xq_sb")
                nc.gpsimd.dma_start(xq_sb[:, :], xq_d.ap())
                for c0 in range(0, MS, 1024):
                    cw = min(1024, MS - c0)
                    ps = mp.tile([64, 1024], F32, tag="s", name=f"q{c0}")
                    for h0 in range(c0, c0 + cw, 512):
                        hw = min(512, c0 + cw - h0)
                        lo = h0 - c0
                        nc.tensor.matmul(ps[0:64, lo:lo + hw],
                                         lhsT=w_sb[0:65, WQC:WQC + 64],
                                         rhs=xq_sb[0:65, h0:h0 + hw],
                                         start=True, stop=False)
                        nc.tensor.matmul(ps[0:64, lo:lo + hw],
                                         lhsT=w_sb[:, WQH:WQH + 64],
                                         rhs=xH_sb[:, h0:h0 + hw],
                                         start=False, stop=True)
                    psum_to_sbuf(q0_sb[0:64, c0:c0 + cw], ps[0:64, 0:cw])
                nc.scalar.dma_start(q0_sb[64:128, :], q0_sb[0:64, :])

                # --- k conv: center + offset-slice D taps, no bias ---
                xin_sb = pr.tile([65, N], BF16, tag="xin", name="xin_sb")
                for p0 in range(0, N, PIECE):
                    pw = min(PIECE, N - p0)
                    nc.sync.dma_start(xin_sb[:, p0:p0 + pw],
                                      xin_d.ap()[:, p0:p0 + pw])
                xW_sb = pr.tile([128, N], BF16, tag="xw", name="xW_sb")
                for p0 in range(0, N, PIECE):
                    pw = min(PIECE, N - p0)
                    nc.scalar.dma_start(xW_sb[:, p0:p0 + pw],
                                        xW_d.ap()[:, p0:p0 + pw])

                def k_half(ps, lo, h0, hw):
                    # accumulation group = one 512-col psum bank
                    nc.tensor.matmul(ps[0:64, lo:lo + hw],
                                     lhsT=w_sb[0:64, WKC:WKC + 64],
                                     rhs=xin_sb[0:64, h0:h0 + hw],
                                     start=True, stop=False)
                    pw = min(h0 + hw, N - DSH) - h0
                    ms = max(h0, DSH)
                    mw_ = h0 + hw - ms
                    if pw > 0:
                        nc.tensor.matmul(
                            ps[0:64, lo:lo + pw],
                            lhsT=w_sb[0:64, WKP:WKP + 64],
                            rhs=xin_sb[0:64, h0 + DSH:h0 + DSH + pw],
                            start=False, stop=(mw_ <= 0))
                    if mw_ > 0:
                        nc.tensor.matmul(
                            ps[0:64, lo + ms - h0:lo + ms - h0 + mw_],
                            lhsT=w_sb[0:64, WKM:WKM + 64],
                            rhs=xin_sb[0:64, ms - DSH:ms - DSH + mw_],
                            start=False, stop=True)

                for c0 in range(0, N, 1024):
                    cw = min(1024, N - c0)
                    ps = mp.tile([64, 1024], F32, tag="s", name=f"k{c0}")
                    for h0 in range(c0, c0 + cw, 512):
                        k_half(ps, h0 - c0, h0, min(512, c0 + cw - h0))
                    psum_to_sbuf(k0_sb[0:64, c0:c0 + cw], ps[0:64, 0:cw])
                    nc.gpsimd.dma_start(k0_sb[64:128, c0:c0 + cw],
                                        k0_sb[0:64, c0:c0 + cw])

                def scores_exp(m0, mw, t, rows, name):
                    chunks = [(0, 512), (512, mw - 512)]
                    s = mp.tile([128, mw], F32, tag="s", name=f"s{name}")
                    for (c0, cw) in chunks:
                        nc.tensor.matmul(
                            s[0:rows, c0:c0 + cw],
                            lhsT=k0_sb[t % 2 * 64:t % 2 * 64 + 64,
                                       t * ITILE:t * ITILE + rows],
                            rhs=q0_sb[t % 2 * 64:t % 2 * 64 + 64,
                                      m0 + c0:m0 + c0 + cw],
                            start=True, stop=True)
                    e = ep.tile([128, mw], BF16, tag="e", name=f"e{name}")
                    if t % 10 in CONFIG["dve_res"]:
                        nc.vector.tensor_scalar(
                            out=e[0:rows, :].bitcast(I16), in0=s[0:rows, :],
                            scalar1=SCH_A, scalar2=SCH_B, op0=MULT, op1=ADD)
                    else:
                        nc.scalar.activation(e[0:rows, :], s[0:rows, :], EXP,
                                             bias=expb[0:rows, :])
                    return e

                # hoisted pass-1 head: scores+exp run while the v conv and
                # transposes occupy the PE, keeping ACT/DVE busy in prep
                hoisted = []
                for t in range(CONFIG["hoist"]):
                    hoisted.append((t, scores_exp(0, 1024, t, ITILE, f"h{t}")))

                # --- v conv (bias ones-row; W taps host-stacked) ---
                for c0 in range(0, N, 1024):
                    cw = min(1024, N - c0)
                    ps = mp.tile([64, 1024], F32, tag="s", name=f"v{c0}")
                    for h0 in range(c0, c0 + cw, 512):
                        hw = min(512, c0 + cw - h0)
                        lo = h0 - c0
                        nc.tensor.matmul(ps[0:64, lo:lo + hw],
                                         lhsT=w_sb[0:65, WVC:WVC + 64],
                                         rhs=xin_sb[0:65, h0:h0 + hw],
                                         start=True, stop=False)
                        nc.tensor.matmul(ps[0:64, lo:lo + hw],
                                         lhsT=w_sb[:, WVW:WVW + 64],
                                         rhs=xW_sb[:, h0:h0 + hw],
                                         start=False, stop=True)
                    psum_to_sbuf(v_sb[0:64, c0:c0 + cw], ps[0:64, 0:cw])

                # --- vT: identity transposes, 8 per psum bank, 1 cast each ---
                for t0 in range(0, NT, 8):
                    ng = min(8, NT - t0)
                    tp = mp.tile([128, 512], F32, tag="s", name=f"tp{t0}")
                    for j in range(ng):
                        t = t0 + j
                        tw = ITILE if t < NFULL else TAILW
                        nc.tensor.matmul(
                            tp[0:tw, j * 64:j * 64 + 64],
                            lhsT=v_sb[0:64, t * ITILE:t * ITILE + tw],
                            rhs=identb[:, :], start=True, stop=True)
                    nc.vector.tensor_copy(
                        out=vT_view[0:128, t0:t0 + ng, 0:64],
                        in_=tp[0:128, 0:ng * 64])

                # --- main attention loop, software-pipelined ---
                for (m0, mw) in passes:
                    poa = opp.tile([65, mw], F32, tag="poa", name=f"poa{m0}")
                    chunks = [(0, 512), (512, mw - 512)]
                    started = {0: False, 512: False}

                    def emit(t, e, rows, last=False):
                        for (c0, cw) in chunks:
                            st = not started[c0]
                            started[c0] = True
                            nc.tensor.matmul(poa[:, c0:c0 + cw],
                                             lhsT=vT_view[0:rows, t, 0:65],
                                             rhs=e[0:rows, c0:c0 + cw],
                                             start=st, stop=last)

                    pend = []
                    if m0 == 0:
                        for (t, e) in hoisted:
                            emit(t, e, ITILE)
                        p_start = CONFIG["hoist"]
                    else:
                        p_start = 0
                    for t in range(p_start, NFULL):
                        pend.append((t, scores_exp(m0, mw, t, ITILE,
                                                   f"t{m0}_{t}")))
                        while len(pend) > 4:
                            t0_, e0 = pend.pop(0)
                            emit(t0_, e0, ITILE)
                    eT = scores_exp(m0, mw, NFULL, TAILW, f"T{m0}")
                    for (t0_, e0) in pend:
                        emit(t0_, e0, ITILE)
                    pend = []
                    emit(NFULL, eT, TAILW, last=True)

                    # normalize: out = num * approx(1/den)
                    rc = fin.tile([1, mw], F32, tag="rc", name=f"rc{m0}")
                    nc.vector.reciprocal(rc[:, :], poa[64:65, 0:mw])
                    bc = fin.tile([64, mw], F32, tag="bc", name=f"bc{m0}")
                    nc.gpsimd.partition_broadcast(bc[:, :], rc[:, :],
                                                  channels=64)
                    nc.vector.tensor_tensor(out=out_sb[0:64, m0:m0 + mw],
                                            in0=poa[0:64, 0:mw], in1=bc[:, :],
                                            op=MULT)
                    nc.sync.dma_start(out_d.ap()[:, m0:m0 + mw],
                                      out_sb[:, m0:m0 + mw])
    nc.compile()
    return nc


def _pack_weights(q_w, k_w, v_w, q_b, v_b):
    """[128, WCOLS] bf16 lhsT pack; see column-offset constants. Bias rows
    (row 64) on the q/v center blocks pair with ones rows in xq/xin."""
    kw = k_w[:, :, :, 0, 0]   # [O, I, 3] taps along D
    qw = q_w[:, :, 0, :, 0]   # taps along H
    vw = v_w[:, :, 0, 0, :]   # taps along W
    wp = np.zeros((128, WCOLS), np.float32)
    wp[0:64, WKC:WKC + 64] = kw[:, :, 1].T
    wp[0:64, WKP:WKP + 64] = kw[:, :, 2].T   # pairs with xin[., n+400]
    wp[0:64, WKM:WKM + 64] = kw[:, :, 0].T   # pairs with xin[., n-400]
    wp[0:64, WQC:WQC + 64] = qw[:, :, 1].T
    wp[64, WQC:WQC + 64] = q_b
    wp[0:64, WQH:WQH + 64] = qw[:, :, 2].T   # rows 0:64 <-> sHp
    wp[64:128, WQH:WQH + 64] = qw[:, :, 0].T
    wp[0:64, WVC:WVC + 64] = vw[:, :, 1].T
    wp[64, WVC:WVC + 64] = v_b
    wp[0:64, WVW:WVW + 64] = vw[:, :, 2].T   # rows 0:64 <-> sWp
    wp[64:128, WVW:WVW + 64] = vw[:, :, 0].T
    return wp.astype(NPB)


def _shifted_hw(xb):
    """xb [C,D,H,W] -> zero-padded unit shifts along H and W, flat [C,N]."""
    z = np.zeros_like(xb)
    sHp = z.copy(); sHp[:, :, :-1] = xb[:, :, 1:]
    sHm = z.copy(); sHm[:, :, 1:] = xb[:, :, :-1]
    sWp = z.copy(); sWp[..., :-1] = xb[..., 1:]
    sWm = z.copy(); sWm[..., 1:] = xb[..., :-1]
    f = lambda a: a.reshape(C, N)
    return f(sHp), f(sHm), f(sWp), f(sWm)


def make_in_maps(x, q_w, q_b, k_w, k_b, v_w, v_b):
    x = np.asarray(x, np.float32)
    wp = _pack_weights(np.asarray(q_w, np.float32),
                       np.asarray(k_w, np.float32),
                       np.asarray(v_w, np.float32),
                       np.asarray(q_b, np.float32),
                       np.asarray(v_b, np.float32))
    per_batch = []
    for b in range(B):
        xb = x[b]
        x2 = xb.reshape(C, N)
        sHp, sHm, sWp, sWm = _shifted_hw(xb)
        xin = np.ones((65, N), np.float32)
        xin[0:64] = x2
        xW = np.vstack([sWp, sWm])
        per_batch.append((x2, xin.astype(NPB), xW.astype(NPB), sHp, sHm))
    in_maps = []
    for g in range(NCORES):
        b, s = g // CPB, g % CPB
        x2, xin, xW, sHp, sHm = per_batch[b]
        off = s * MS
        xq = np.ones((65, MS), np.float32)
        xq[0:64] = x2[:, off:off + MS]
        xH = np.vstack([sHp[:, off:off + MS], sHm[:, off:off + MS]])
        in_maps.append({
            "xin": xin,
            "xW": xW,
            "xH": np.ascontiguousarray(xH.astype(NPB)),
            "xq": np.ascontiguousarray(xq.astype(NPB)),
            "wp": wp,
        })
    return in_maps


def kernel(x, q_w, q_b, k_w, k_b, v_w, v_b, trace=False):
    from concourse.bass_utils import run_bass_kernel_spmd
    if "nc" not in _CACHE:
        _CACHE["nc"] = _build_bass()
    nc = _CACHE["nc"]
    in_maps = make_in_maps(x, q_w, q_b, k_w, k_b, v_w, v_b)
    res = run_bass_kernel_spmd(nc, in_maps, core_ids=list(range(NCORES)),
                               trace=trace)
    _CACHE["last_result"] = res
    out = np.empty((B, C, N), np.float32)
    for g in range(NCORES):
        b, s = g // CPB, g % CPB
        out[b, :, s * MS:(s + 1) * MS] = res.results[g]["out"]
    return out.reshape(B, C, D, H, W)


# revision 6
# speedup vs baseline: 1.2247x; 1.2247x over previous
"""DualAttention Trainium2 kernel.

Problem: x:[2,64,20,20,20]; three separable 1-D convs produce q0 (H-axis),
k0 (D-axis), v (W-axis), each [B,C,N] with N=8000; scores = k0^T q0 [B,N,N];
softmax over the key axis i (axis 1); out = v @ attn, reshaped back.

Sharding: 8 cores = 2 batches x 4 query-column slices of 2000. Each core
computes full k0/v (cheap convs) and its q0 slice, then a flash-style
scores->exp->accumulate loop. No collectives.

Per-core device algorithm (Tile framework), v3 (trace-driven):
  - all conv inputs/weights bf16 (full-rate PE, half the DMA bytes).
  - D-axis taps of the k conv are +/-400-column OFFSET slices of xin
    (valid region contiguous in flattened N) -> no host-stacked xD tensor.
  - k bias dropped entirely (adds a per-query-column constant to scores
    -> cancels in softmax over the key axis); q/v biases ride free on the
    PSUM->SBUF conv casts (ACT bias / DVE tensor_scalar add), which
    alternate between the two engines.
  - ~16 dummy warmup matmuls issued first each iteration keep the PE HAM
    un-throttled through the input-DMA window.
  - scores: bf16 k0/q0 (fp32 PSUM); i-tile pairs alternate PE row groups
    (k0/q0 duplicated across partition halves) so adjacent tiles stream
    concurrently and LDWEIGHTS pulls ahead.
  - exp with bias -8 (uniform shift cancels in softmax), bf16 out; half
    of i-tiles on ScalarE (exact), half on VectorE via a Schraudolph-style
    int16 tensor_scalar whose bits are the bf16 exp approximation.
  - vT built with identity matmuls, 8 transposes packed per PSUM bank,
    one DVE cast per group.
  - second matmul: K=128 contraction split into two K=64 row-group halves
    (psum banks poa/pob, merged at normalize). NOTE: a single K=128 MM is
    cycle-equivalent in theory but measured 373 ns vs 216+overlap — a
    full-array MM blocks the next LDWEIGHTS from pulling ahead, so every
    out-MM pays fill+drain; the row-group split hides it.
  - normalize: merge halves, reciprocal of the den row, gpsimd partition
    broadcast, multiply, per-pass output DMA.
"""

import math
import numpy as np
import ml_dtypes

B, C, D, H, W = 2, 64, 20, 20, 20
N = D * H * W            # 8000
NCORES = 8
CPB = 4                  # cores per batch
MS = N // CPB            # 2000 query columns per core
ITILE = 128
NFULL = N // ITILE       # 62 full i-tiles
TAILW = N - NFULL * ITILE  # 64
NT = NFULL + 1           # 63
M_PASSES = [(0, 1024), (1024, 976)]
EXP_BIAS = -8.0
DSH = H * W              # 400: one D step in flattened N
NPB = np.dtype(ml_dtypes.bfloat16)

# Schraudolph-style exp on DVE: int16 bits = SCH_A*s + SCH_B viewed as bf16.
# bf16 bits = 128*log2(v) + 16256; log2(e^(s+EXP_BIAS)) = (s+EXP_BIAS)/ln2.
SCH_A = 128.0 / math.log(2.0)
SCH_C = 7.4
SCH_B = 16256.0 - SCH_C + SCH_A * EXP_BIAS

# build-time config knobs (A/B testing)
CONFIG = {
    "dve_res": (1, 3, 5, 7, 9),  # t%10 in set -> exp on DVE (50%)
    "warm_mms": 16,
    "hoist": 8,
    "mp_bufs": 2,
    "ep_bufs": 12,
    "hint": True,
}

_CACHE = {}

# weight-pack column offsets (all blocks 64 wide)
WKC, WKP, WKM, WQC, WQH, WVC, WVW = 0, 64, 128, 192, 256, 320, 384
WCOLS = 448


def _build_bass(reps=1, bench_loop=0, passes=None):
    if passes is None:
        passes = M_PASSES
    import contextlib
    import concourse.tile as tile
    import concourse.mybir as mybir
    from concourse import bacc
    from concourse.masks import make_identity

    F32 = mybir.dt.float32
    I16 = mybir.dt.int16
    BF16 = mybir.dt.bfloat16
    EXP = mybir.ActivationFunctionType.Exp
    IDENT = mybir.ActivationFunctionType.Identity
    MULT = mybir.AluOpType.mult
    ADD = mybir.AluOpType.add
    ET = mybir.EngineType

    nc = bacc.Bacc("TRN2", target_bir_lowering=False, debug=False,
                   num_devices=NCORES)

    xin_d = nc.dram_tensor("xin", [64, N], BF16, kind="ExternalInput")
    xW_d = nc.dram_tensor("xW", [128, N], BF16, kind="ExternalInput")
    xH_d = nc.dram_tensor("xH", [128, MS], BF16, kind="ExternalInput")
    xq_d = nc.dram_tensor("xq", [64, MS], BF16, kind="ExternalInput")
    wp_d = nc.dram_tensor("wp", [128, WCOLS], BF16, kind="ExternalInput")
    bp_d = nc.dram_tensor("bp", [64, 2], F32, kind="ExternalInput")
    out_d = nc.dram_tensor("out", [64, MS], F32, kind="ExternalOutput")

    PIECE = 2048  # input DMA piece size for pipelining

    with tile.TileContext(nc) as tc:
        hint = tuple()
        if bench_loop and CONFIG["hint"]:
            hint = (ET.PE, ET.Activation, ET.DVE)
        loop_cm = (tc.For_i(0, bench_loop, 1, hint_engines=hint)
                   if bench_loop else contextlib.nullcontext())
        with loop_cm:
         for _rep in range(reps):
            with tc.tile_pool(name="persist", bufs=1) as per, \
                 tc.tile_pool(name="mps", bufs=CONFIG["mp_bufs"],
                              space="PSUM") as mp, \
                 tc.tile_pool(name="ops", bufs=1, space="PSUM") as opp, \
                 tc.tile_pool(name="eps", bufs=CONFIG["ep_bufs"]) as ep, \
                 tc.tile_pool(name="fin", bufs=1) as fin, \
                 tc.tile_pool(name="prep", bufs=1) as pr:

                w_sb = per.tile([128, WCOLS], BF16, name="w_sb")
                nc.gpsimd.dma_start(w_sb[:, :], wp_d.ap())
                b_sb = per.tile([64, 2], F32, name="b_sb")
                nc.gpsimd.dma_start(b_sb[:, :], bp_d.ap())
                k0_sb = per.tile([128, N], BF16, name="k0_sb")
                q0_sb = per.tile([128, MS], BF16, name="q0_sb")
                v_sb = per.tile([64, N], BF16, name="v_sb")
                vT_sb = per.tile([128, NT * 65], BF16, name="vT_sb")
                out_sb = per.tile([64, MS], F32, name="out_sb")
                identb = per.tile([64, 64], BF16, name="identb")
                expb = per.tile([128, 1], F32, name="expb")
                wsrc = per.tile([64, 512], BF16, name="wsrc")

                make_identity(nc, identb[:, :])
                nc.vector.memset(expb[:, :], EXP_BIAS)
                nc.vector.memset(vT_sb[:, :], 1.0)
                nc.gpsimd.memset(wsrc[:, :], 0.0)
                vT_view = vT_sb[:, :].rearrange("p (t c) -> p t c", c=65)

                # --- PE warmup: keep HAM un-throttled during input DMA ---
                warm = mp.tile([64, 512], F32, tag="s", name="warm")
                for _ in range(CONFIG["warm_mms"]):
                    nc.tensor.matmul(warm[0:64, :], lhsT=wsrc[0:64, 0:64],
                                     rhs=wsrc[0:64, :],
                                     start=True, stop=True)

                copy_tick = [0]

                def psum_to_sbuf(dst, src, bcol=None):
                    # alternate ACT/DVE for PSUM->SBUF conv casts; the
                    # q/v bias rides free on either engine's op
                    if copy_tick[0] % 2 == 0:
                        bias = (b_sb[0:64, bcol:bcol + 1]
                                if bcol is not None else 0.0)
                        nc.scalar.activation(dst, src, IDENT, bias=bias)
                    elif bcol is not None:
                        nc.vector.tensor_scalar_add(
                            out=dst, in0=src,
                            scalar1=b_sb[0:64, bcol:bcol + 1])
                    else:
                        nc.vector.tensor_copy(out=dst, in_=src)
                    copy_tick[0] += 1

                # --- q conv first (small inputs, unblocks scores) ---
                xH_sb = pr.tile([128, MS], BF16, tag="xh", name="xH_sb")
                nc.sync.dma_start(xH_sb[:, :], xH_d.ap())
                xq_sb = pr.tile([64, MS], BF16, tag="xq", name="xq_sb")
                nc.gpsimd.dma_start(xq_sb[:, :], xq_d.ap())
                for c0 in range(0, MS, 1024):
                    cw = min(1024, MS - c0)
                    ps = mp.tile([64, 1024], F32, tag="s", name=f"q{c0}")
                    for h0 in range(c0, c0 + cw, 512):
                        hw = min(512, c0 + cw - h0)
                        lo = h0 - c0
                        nc.tensor.matmul(ps[0:64, lo:lo + hw],
                                         lhsT=w_sb[0:64, WQC:WQC + 64],
                                         rhs=xq_sb[0:64, h0:h0 + hw],
                                         start=True, stop=False)
                        nc.tensor.matmul(ps[0:64, lo:lo + hw],
                                         lhsT=w_sb[:, WQH:WQH + 64],
                                         rhs=xH_sb[:, h0:h0 + hw],
                                         start=False, stop=True)
                    psum_to_sbuf(q0_sb[0:64, c0:c0 + cw], ps[0:64, 0:cw],
                                 bcol=0)
                nc.scalar.dma_start(q0_sb[64:128, :], q0_sb[0:64, :])

                # --- k conv: center + offset-slice D taps, no bias ---
                xin_sb = pr.tile([64, N], BF16, tag="xin", name="xin_sb")
                for p0 in range(0, N, PIECE):
                    pw = min(PIECE, N - p0)
                    nc.sync.dma_start(xin_sb[:, p0:p0 + pw],
                                      xin_d.ap()[:, p0:p0 + pw])
                xW_sb = pr.tile([128, N], BF16, tag="xw", name="xW_sb")
                for p0 in range(0, N, PIECE):
                    pw = min(PIECE, N - p0)
                    nc.scalar.dma_start(xW_sb[:, p0:p0 + pw],
                                        xW_d.ap()[:, p0:p0 + pw])

                def k_half(ps, lo, h0, hw):
                    # accumulation group = one 512-col psum bank
                    nc.tensor.matmul(ps[0:64, lo:lo + hw],
                                     lhsT=w_sb[0:64, WKC:WKC + 64],
                                     rhs=xin_sb[0:64, h0:h0 + hw],
                                     start=True, stop=False)
                    pw = min(h0 + hw, N - DSH) - h0
                    ms = max(h0, DSH)
                    mw_ = h0 + hw - ms
                    if pw > 0:
                        nc.tensor.matmul(
                            ps[0:64, lo:lo + pw],
                            lhsT=w_sb[0:64, WKP:WKP + 64],
                            rhs=xin_sb[0:64, h0 + DSH:h0 + DSH + pw],
                            start=False, stop=(mw_ <= 0))
                    if mw_ > 0:
                        nc.tensor.matmul(
                            ps[0:64, lo + ms - h0:lo + ms - h0 + mw_],
                            lhsT=w_sb[0:64, WKM:WKM + 64],
                            rhs=xin_sb[0:64, ms - DSH:ms - DSH + mw_],
                            start=False, stop=True)

                for c0 in range(0, N, 1024):
                    cw = min(1024, N - c0)
                    ps = mp.tile([64, 1024], F32, tag="s", name=f"k{c0}")
                    for h0 in range(c0, c0 + cw, 512):
                        k_half(ps, h0 - c0, h0, min(512, c0 + cw - h0))
                    psum_to_sbuf(k0_sb[0:64, c0:c0 + cw], ps[0:64, 0:cw])
                    nc.gpsimd.dma_start(k0_sb[64:128, c0:c0 + cw],
                                        k0_sb[0:64, c0:c0 + cw])

                def scores_exp(m0, mw, t, rows, name):
                    chunks = [(0, 512), (512, mw - 512)]
                    s = mp.tile([128, mw], F32, tag="s", name=f"s{name}")
                    for (c0, cw) in chunks:
                        nc.tensor.matmul(
                            s[0:rows, c0:c0 + cw],
                            lhsT=k0_sb[t % 2 * 64:t % 2 * 64 + 64,
                                       t * ITILE:t * ITILE + rows],
                            rhs=q0_sb[t % 2 * 64:t % 2 * 64 + 64,
                                      m0 + c0:m0 + c0 + cw],
                            start=True, stop=True)
                    e = ep.tile([128, mw], BF16, tag="e", name=f"e{name}")
                    if t % 10 in CONFIG["dve_res"]:
                        nc.vector.tensor_scalar(
                            out=e[0:rows, :].bitcast(I16), in0=s[0:rows, :],
                            scalar1=SCH_A, scalar2=SCH_B, op0=MULT, op1=ADD)
                    else:
                        nc.scalar.activation(e[0:rows, :], s[0:rows, :], EXP,
                                             bias=expb[0:rows, :])
                    return e

                def out_mms(poa, pob, mw, t, rows, e, first_a, first_b,
                            last_a, last_b):
                    # K=128 contraction split into two K=64 row-group halves
                    # (psum banks poa/pob) so consecutive out-MMs alternate
                    # row groups and LDWEIGHTS/drain overlap.
                    for (c0, cw) in [(0, 512), (512, mw - 512)]:
                        nc.tensor.matmul(poa[:, c0:c0 + cw],
                                         lhsT=vT_view[0:64, t, :],
                                         rhs=e[0:64, c0:c0 + cw],
                                         start=first_a, stop=last_a)
                        if rows > 64:
                            nc.tensor.matmul(pob[:, c0:c0 + cw],
                                             lhsT=vT_view[64:128, t, :],
                                             rhs=e[64:128, c0:c0 + cw],
                                             start=first_b, stop=last_b)

                # hoisted pass-1 head: scores+exp run while the v conv and
                # transposes occupy the PE, keeping ACT/DVE busy in prep
                hoisted = []
                for t in range(CONFIG["hoist"]):
                    hoisted.append((t, scores_exp(0, 1024, t, ITILE, f"h{t}")))

                # --- v conv (W taps host-stacked; bias on the cast) ---
                for c0 in range(0, N, 1024):
                    cw = min(1024, N - c0)
                    ps = mp.tile([64, 1024], F32, tag="s", name=f"v{c0}")
                    for h0 in range(c0, c0 + cw, 512):
                        hw = min(512, c0 + cw - h0)
                        lo = h0 - c0
                        nc.tensor.matmul(ps[0:64, lo:lo + hw],
                                         lhsT=w_sb[0:64, WVC:WVC + 64],
                                         rhs=xin_sb[0:64, h0:h0 + hw],
                                         start=True, stop=False)
                        nc.tensor.matmul(ps[0:64, lo:lo + hw],
                                         lhsT=w_sb[:, WVW:WVW + 64],
                                         rhs=xW_sb[:, h0:h0 + hw],
                                         start=False, stop=True)
                    psum_to_sbuf(v_sb[0:64, c0:c0 + cw], ps[0:64, 0:cw],
                                 bcol=1)

                # --- vT: identity transposes, 8 per psum bank, 1 cast each ---
                for t0 in range(0, NT, 8):
                    ng = min(8, NT - t0)
                    tp = mp.tile([128, 512], F32, tag="s", name=f"tp{t0}")
                    for j in range(ng):
                        t = t0 + j
                        tw = ITILE if t < NFULL else TAILW
                        nc.tensor.matmul(
                            tp[0:tw, j * 64:j * 64 + 64],
                            lhsT=v_sb[0:64, t * ITILE:t * ITILE + tw],
                            rhs=identb[:, :], start=True, stop=True)
                    nc.vector.tensor_copy(
                        out=vT_view[0:128, t0:t0 + ng, 0:64],
                        in_=tp[0:128, 0:ng * 64])

                # --- main attention loop, software-pipelined ---
                for (m0, mw) in passes:
                    poa = opp.tile([65, mw], F32, tag="poa", name=f"poa{m0}")
                    pob = opp.tile([65, mw], F32, tag="pob", name=f"pob{m0}")
                    ema = {"a": False, "b": False}  # emitted-first per bank

                    def emit(t, e, rows, last=False):
                        fa, fb = not ema["a"], not ema["b"]
                        out_mms(poa, pob, mw, t, rows, e, fa, fb,
                                last, last)
                        ema["a"] = True
                        if rows > 64:
                            ema["b"] = True

                    pend = []
                    if m0 == 0:
                        for (t, e) in hoisted:
                            emit(t, e, ITILE)
                        p_start = CONFIG["hoist"]
                    else:
                        p_start = 0
                    for t in range(p_start, NFULL):
                        pend.append((t, scores_exp(m0, mw, t, ITILE,
                                                   f"t{m0}_{t}")))
                        while len(pend) > 4:
                            t0_, e0 = pend.pop(0)
                            emit(t0_, e0, ITILE)
                    eT = scores_exp(m0, mw, NFULL, TAILW, f"T{m0}")
                    for i, (t0_, e0) in enumerate(pend):
                        # bank B's last write is the final full tile
                        fa, fb = not ema["a"], not ema["b"]
                        out_mms(poa, pob, mw, t0_, ITILE, e0, fa, fb,
                                False, i == len(pend) - 1)
                        ema["a"] = ema["b"] = True
                    pend = []
                    emit(NFULL, eT, TAILW, last=True)

                    # merge halves + normalize: out = num * (1/den)
                    nb = fin.tile([65, mw], F32, tag="nb", name=f"nb{m0}")
                    nc.vector.tensor_copy(out=nb[:, :], in_=pob[0:65, 0:mw])
                    ns = fin.tile([65, mw], F32, tag="ns", name=f"ns{m0}")
                    nc.vector.tensor_tensor(out=ns[:, :], in0=poa[0:65, 0:mw],
                                            in1=nb[:, :], op=ADD)
                    rc = fin.tile([1, mw], F32, tag="rc", name=f"rc{m0}")
                    nc.vector.reciprocal(rc[:, :], ns[64:65, :])
                    bc = fin.tile([64, mw], F32, tag="bc", name=f"bc{m0}")
                    nc.gpsimd.partition_broadcast(bc[:, :], rc[:, :],
                                                  channels=64)
                    nc.vector.tensor_tensor(out=out_sb[0:64, m0:m0 + mw],
                                            in0=ns[0:64, :], in1=bc[:, :],
                                            op=MULT)
                    nc.sync.dma_start(out_d.ap()[:, m0:m0 + mw],
                                      out_sb[:, m0:m0 + mw])
    nc.compile()
    return nc


def _pack_weights(q_w, k_w, v_w):
    """[128, WCOLS] bf16 lhsT pack; see column-offset constants."""
    kw = k_w[:, :, :, 0, 0]   # [O, I, 3] taps along D
    qw = q_w[:, :, 0, :, 0]   # taps along H
    vw = v_w[:, :, 0, 0, :]   # taps along W
    wp = np.zeros((128, WCOLS), np.float32)
    wp[0:64, WKC:WKC + 64] = kw[:, :, 1].T
    wp[0:64, WKP:WKP + 64] = kw[:, :, 2].T   # pairs with xin[., n+400]
    wp[0:64, WKM:WKM + 64] = kw[:, :, 0].T   # pairs with xin[., n-400]
    wp[0:64, WQC:WQC + 64] = qw[:, :, 1].T
    wp[0:64, WQH:WQH + 64] = qw[:, :, 2].T   # rows 0:64 <-> sHp
    wp[64:128, WQH:WQH + 64] = qw[:, :, 0].T
    wp[0:64, WVC:WVC + 64] = vw[:, :, 1].T
    wp[0:64, WVW:WVW + 64] = vw[:, :, 2].T   # rows 0:64 <-> sWp
    wp[64:128, WVW:WVW + 64] = vw[:, :, 0].T
    return wp.astype(NPB)


def _shifted_hw(xb):
    """xb [C,D,H,W] -> zero-padded unit shifts along H and W, flat [C,N]."""
    z = np.zeros_like(xb)
    sHp = z.copy(); sHp[:, :, :-1] = xb[:, :, 1:]
    sHm = z.copy(); sHm[:, :, 1:] = xb[:, :, :-1]
    sWp = z.copy(); sWp[..., :-1] = xb[..., 1:]
    sWm = z.copy(); sWm[..., 1:] = xb[..., :-1]
    f = lambda a: a.reshape(C, N)
    return f(sHp), f(sHm), f(sWp), f(sWm)


def make_in_maps(x, q_w, q_b, k_w, k_b, v_w, v_b):
    x = np.asarray(x, np.float32)
    wp = _pack_weights(np.asarray(q_w, np.float32),
                       np.asarray(k_w, np.float32),
                       np.asarray(v_w, np.float32))
    bp = np.stack([np.asarray(q_b, np.float32),
                   np.asarray(v_b, np.float32)], axis=1)  # [64, 2]
    per_batch = []
    for b in range(B):
        xb = x[b]
        x2 = xb.reshape(C, N)
        sHp, sHm, sWp, sWm = _shifted_hw(xb)
        xW = np.vstack([sWp, sWm])
        per_batch.append((x2.astype(NPB), xW.astype(NPB), sHp, sHm))
    in_maps = []
    for g in range(NCORES):
        b, s = g // CPB, g % CPB
        x2, xW, sHp, sHm = per_batch[b]
        off = s * MS
        xH = np.vstack([sHp[:, off:off + MS], sHm[:, off:off + MS]])
        in_maps.append({
            "xin": x2,
            "xW": xW,
            "xH": np.ascontiguousarray(xH.astype(NPB)),
            "xq": np.ascontiguousarray(x2[:, off:off + MS]),
            "wp": wp,
            "bp": bp,
        })
    return in_maps


def kernel(x, q_w, q_b, k_w, k_b, v_w, v_b, trace=False):
    from concourse.bass_utils import run_bass_kernel_spmd
    if "nc" not in _CACHE:
        _CACHE["nc"] = _build_bass()
    nc = _CACHE["nc"]
    in_maps = make_in_maps(x, q_w, q_b, k_w, k_b, v_w, v_b)
    res = run_bass_kernel_spmd(nc, in_maps, core_ids=list(range(NCORES)),
                               trace=trace)
    _CACHE["last_result"] = res
    out = np.empty((B, C, N), np.float32)
    for g in range(NCORES):
        b, s = g // CPB, g % CPB
        out[b, :, s * MS:(s + 1) * MS] = res.results[g]["out"]
    return out.reshape(B, C, D, H, W)
